# revision 79
# baseline (speedup 1.0000x reference)
"""Causal GQA self-attention (RoPE) Trainium2 Bass kernel, 8-core SPMD.

Sharding: core c -> (b = c//4, g = c%4).  Data-parallel over batch B=2,
tensor-parallel over the 4 KV groups (4 query heads + 1 KV head each).
Each core computes a partial output y_bg = attn_out_g @ Wo[:, g-block].T
for its batch; the host sums the 4 group partials per batch (row-parallel
linear unshard).  Partials are returned in bf16 and summed in f32.

Per-core device kernel (all matmuls bf16, f32 PSUM accumulation),
software-pipelined over 512-wide l-chunks (lc = query chunk = row group):
  x arrives pre-transposed from the host as [128, lc, dti, 512] bf16.
  Projections: 16 dti-accum matmuls into PSUM per (lc, tensor); PSUM->SBUF
    on the scalar engine; RoPE rotate-half via a signed-perm matmul + DVE
    cos/sin multiplies; V to [keys, hd] layout via XBAR DMA-transpose.
  Attention for qi=lc: per (h, kt): S^T block matmul, Exp on ACT (scale
    folded), causal diag mask on DVE, softmax denominators via GpSimd
    partition_all_reduce (+ DVE accum) -- except the last chunk, where
    a third of the key tiles use a PE ones-matmul instead (the last
    chunk has no projection work left to keep the PE busy); attn@V
    accumulated in PSUM, with heads processed in concurrent pairs and a
    one-key-tile emission skew so the PE never waits on Exp.
  o-proj per 128-row group accumulated over heads; PSUM->SBUF on DVE;
    row-group DMA stores of the bf16 partial.
  Emission order interleaves attention of chunk lc with projections of
  lc+1 and o-proj of lc-1 so the in-order PE queue always has
  dependency-free matmuls available.
"""

import math
import sys

import numpy as np

try:
    import concourse.bass as bass  # noqa: F401
except ImportError:  # pragma: no cover
    sys.path.insert(0, "/opt/trn_rl_repo")
    import concourse.bass as bass  # noqa: F401

import ml_dtypes

import concourse.bacc as bacc
import concourse.bass_isa as bass_isa
import concourse.mybir as mybir
import concourse.tile as tile
from concourse.bass_utils import run_bass_kernel_spmd

BF16 = ml_dtypes.bfloat16
F32 = np.float32

B, L, D = 2, 2048, 2048
HD = 128          # head dim
NHL = 4           # query heads per core (one KV group)
P = 128
NDT = D // P      # 16 d-tiles
NKT = L // P      # 16 key tiles
NLC = L // 512    # 4 512-wide l chunks
SM_SCALE = 1.0 / math.sqrt(HD)

_BF = mybir.dt.bfloat16
_F32 = mybir.dt.float32

# per-lc projection emission order: earliest consumers first
ET_ORDER = ("k", "v", "q0", "q1", "q2", "q3")


def build_nc():
    nc = bacc.Bacc("TRN2", target_bir_lowering=False, debug=False,
                   enable_asserts=False)

    xh_d = nc.dram_tensor("xh", [P, NLC, NDT, 512], _BF, kind="ExternalInput").ap()
    wq_d = nc.dram_tensor("wq", [P, NDT, 512], _BF, kind="ExternalInput").ap()
    wk_d = nc.dram_tensor("wk", [P, NDT, 128], _BF, kind="ExternalInput").ap()
    wv_d = nc.dram_tensor("wv", [P, NDT, 128], _BF, kind="ExternalInput").ap()
    wo_d = nc.dram_tensor("wo", [P, NHL, L], _BF, kind="ExternalInput").ap()
    cos_d = nc.dram_tensor("cosT", [P, L], _BF, kind="ExternalInput").ap()
    sin_d = nc.dram_tensor("sinT", [P, L], _BF, kind="ExternalInput").ap()
    perm_d = nc.dram_tensor("perm", [P, P], _BF, kind="ExternalInput").ap()
    tri_d = nc.dram_tensor("tri", [P, P], _BF, kind="ExternalInput").ap()
    ones_d = nc.dram_tensor("ones", [P, P], _BF, kind="ExternalInput").ap()
    y_d = nc.dram_tensor("y", [L, D], _BF, kind="ExternalOutput").ap()

    with tile.TileContext(nc) as tc:
        _body(nc, tc, xh_d, wq_d, wk_d, wv_d, wo_d, cos_d, sin_d, perm_d,
              tri_d, ones_d, y_d)
    nc.compile()
    return nc


def _body(nc, tc, xh_d, wq_d, wk_d, wv_d, wo_d, cos_d, sin_d, perm_d,
          tri_d, ones_d, y_d):
    from contextlib import ExitStack
    ctx = ExitStack()
    with ctx:
        pp = ctx.enter_context(tc.tile_pool(name="persist", bufs=1))
        wsb = ctx.enter_context(tc.tile_pool(name="wsb", bufs=2))
        psp = ctx.enter_context(tc.tile_pool(name="psp", bufs=1, space="PSUM"))

        xsbA = pp.tile([P, NLC, 8, 512], _BF, tag="xsbA")
        xsbB = pp.tile([P, NLC, 8, 512], _BF, tag="xsbB")
        wq_sb = pp.tile([P, NDT, 512], _BF, tag="wq")
        wk_sb = pp.tile([P, NDT, 128], _BF, tag="wk")
        wv_sb = pp.tile([P, NDT, 128], _BF, tag="wv")
        wo_sb = pp.tile([P, NHL, L], _BF, tag="wo")
        cos_sb = pp.tile([P, L], _BF, tag="cos")
        sin_sb = pp.tile([P, L], _BF, tag="sin")
        perm_sb = pp.tile([P, P], _BF, tag="perm")
        tri_sb = pp.tile([P, P], _BF, tag="tri")
        ones_sb = pp.tile([P, P], _BF, tag="ones")
        qT = pp.tile([P, NHL, L], _BF, tag="qT")
        kT = pp.tile([P, L], _BF, tag="kT")
        vn = pp.tile([P, NKT, 128], _BF, tag="vn")
        oT = pp.tile([P, NHL, L], _BF, tag="oT")

        # input DMAs, ordered to match first-consumer order on the PE;
        # the big late chunks (x2, x3, wo) are issued just-in-time from
        # attention slots so small latency-critical DMAs aren't queued
        # behind them on the serial DMA engines.
        nc.sync.dma_start(wk_sb[:], wk_d[:])
        nc.sync.dma_start(xsbA[:, 0, 0:2], xh_d[:, 0, 0:2])
        nc.sync.dma_start(xsbA[:, 0, 2:4], xh_d[:, 0, 2:4])
        nc.sync.dma_start(xsbA[:, 0, 4:8], xh_d[:, 0, 4:8])
        nc.sync.dma_start(xsbB[:, 0, 0:4], xh_d[:, 0, 8:12])
        nc.sync.dma_start(xsbB[:, 0, 4:8], xh_d[:, 0, 12:16])
        nc.sync.dma_start(perm_sb[:], perm_d[:])
        nc.sync.dma_start(wv_sb[:], wv_d[:])
        nc.sync.dma_start(wq_sb[:, :, 0:256], wq_d[:, :, 0:256])
        nc.sync.dma_start(wq_sb[:, :, 256:512], wq_d[:, :, 256:512])
        nc.sync.dma_start(cos_sb[:], cos_d[:])
        nc.sync.dma_start(sin_sb[:], sin_d[:])
        nc.sync.dma_start(tri_sb[:], tri_d[:])
        nc.sync.dma_start(ones_sb[:], ones_d[:])
        nc.sync.dma_start(xsbA[:, 1], xh_d[:, 1, 0:8])
        nc.sync.dma_start(xsbB[:, 1], xh_d[:, 1, 8:16])
        deferred = {
            (0, 0): [(wo_sb[:, 0:2], wo_d[:, 0:2]),
                     (xsbA[:, 2], xh_d[:, 2, 0:8])],
            (0, 1): [(xsbB[:, 2], xh_d[:, 2, 8:16])],
            (1, 0): [(wo_sb[:, 2:4], wo_d[:, 2:4]),
                     (xsbA[:, 3], xh_d[:, 3, 0:8])],
            (2, 0): [(xsbB[:, 3], xh_d[:, 3, 8:16])],
        }

        def gen_proj(lc, et):
            """Generator: yields after each matmul so projection work can be
            interleaved into attention rounds at matmul granularity."""
            ls = slice(lc * 512, (lc + 1) * 512)
            if et == "k":
                w_sl = lambda d_: wk_sb[:, d_, :]
            elif et == "v":
                w_sl = lambda d_: wv_sb[:, d_, :]
            else:
                h_ = int(et[1])
                w_sl = lambda d_: wq_sb[:, d_, h_ * 128:(h_ + 1) * 128]

            prj = psp.tile([P, 512], _F32, tag="prj", bufs=2,
                           name=f"prj_{lc}_{et}")
            for dti in range(NDT):
                xs = (xsbA[:, lc, dti, :] if dti < 8
                      else xsbB[:, lc, dti - 8, :])
                nc.tensor.matmul(prj[:], w_sl(dti), xs,
                                 start=(dti == 0), stop=(dti == NDT - 1),
                                 skip_group_check=True)
                yield

            qsb = wsb.tile([P, 512], _BF, tag="qsb", bufs=4,
                           name=f"qsb_{lc}_{et}")
            nc.scalar.activation(qsb[:], prj[:],
                                 mybir.ActivationFunctionType.Copy)
            if et == "v":
                # v chunk -> vn[keys, hd] via XBAR DMA transpose
                for j in range(4):
                    nc.sync.dma_start_transpose(
                        vn[:, 4 * lc + j, :], qsb[:, j * 128:(j + 1) * 128])
            else:
                dest = kT[:, ls] if et == "k" else qT[:, int(et[1]), ls]
                # rotate-half across partitions via signed-perm matmul
                # (parks in the py0 o-proj ring, which is never active
                # at the same time as a projection tail)
                qrot = psp.tile([P, 512], _F32, tag="py0", bufs=1,
                                name=f"qrot_{lc}_{et}")
                nc.tensor.matmul(qrot[:], perm_sb[:], qsb[:],
                                 start=True, stop=True)
                yield
                tt = wsb.tile([P, 512], _BF, tag="tt", bufs=4,
                              name=f"tt_{lc}_{et}")
                nc.vector.tensor_mul(tt[:], qsb[:], cos_sb[:, ls])
                nc.vector.tensor_mul(dest, qrot[:], sin_sb[:, ls])
                nc.vector.tensor_add(dest, dest, tt[:])

        def emit_att_pair(lc, h0, pool):
            """Attention for heads (h0, h0+1) processed concurrently per
            key tile, so the PE round S/S'/AV/AV' hides the Exp latency."""
            nvis = 4 * lc
            nkt = nvis + 4
            q0 = lc * 512
            ls = slice(q0, q0 + 512)
            hs = (h0, h0 + 1)
            # last chunk: every third key tile uses a PE ones-matmul
            # denominator (no projection work remains to fill the PE)
            if lc == NLC - 1:
                pe_kt = lambda kt: kt % 3 == 2
            else:
                pe_kt = lambda kt: False
            pe_list = [kt for kt in range(nkt) if pe_kt(kt)]

            po = {h: psp.tile([P, 512], _F32, tag="po", bufs=2,
                              name=f"po_{lc}_{h}") for h in hs}
            acc = {h: wsb.tile([P, 512], _F32, tag="acc", bufs=2,
                               name=f"acc_{lc}_{h}") for h in hs}
            psum_sum = {}
            if pe_list:
                if h0 == 2:
                    # second pair of the last chunk: py rings are idle
                    # (no proj fillers, boundary o-projs already emitted),
                    # and keeping prj free unblocks the final o-proj
                    psum_sum = {hs[0]: psp.tile([P, 512], _F32, tag="py0",
                                                bufs=1,
                                                name=f"sum_{lc}_{hs[0]}"),
                                hs[1]: psp.tile([P, 512], _F32, tag="py1",
                                                bufs=1,
                                                name=f"sum_{lc}_{hs[1]}")}
                else:
                    psum_sum = {h: psp.tile([P, 512], _F32, tag="prj",
                                            bufs=2, name=f"sum_{lc}_{h}")
                                for h in hs}
            ess = {}

            def denom(h, kt):
                off = max(0, (kt - nvis) * P)
                cs = slice(off, 512)
                nc.tensor.matmul(
                    psum_sum[h][:, cs], ones_sb[:], ess[h, kt][:, cs],
                    start=(kt == pe_list[0]), stop=(kt == pe_list[-1]),
                    skip_group_check=True)

            def av(h, kt):
                off = max(0, (kt - nvis) * P)
                cs = slice(off, 512)
                nc.tensor.matmul(
                    po[h][:, cs], vn[:, kt, :], ess[h, kt][:, cs],
                    start=(kt == 0), stop=(kt == nkt - 1),
                    skip_group_check=True)

            for kt in range(nkt):
                off = max(0, (kt - nvis) * P)
                cs = slice(off, 512)
                diag = kt >= nvis
                for h in hs:
                    ps = psp.tile([P, 512], _F32, tag="ps", bufs=2,
                                  name=f"ps_{lc}_{h}_{kt}")
                    nc.tensor.matmul(
                        ps[:, cs], kT[:, kt * P:(kt + 1) * P],
                        qT[:, h, q0 + off:q0 + 512],
                        start=True, stop=True, skip_group_check=True)
                    es = wsb.tile([P, 512], _BF, tag="es", bufs=12,
                                  name=f"es_{lc}_{h}_{kt}")
                    ess[h, kt] = es
                    nc.scalar.activation(
                        es[:, cs], ps[:, cs],
                        mybir.ActivationFunctionType.Exp, scale=SM_SCALE)
                    if diag:
                        nc.vector.tensor_mul(es[:, off:off + P],
                                             es[:, off:off + P], tri_sb[:])
                    if not pe_kt(kt):
                        if kt == 0:
                            nc.gpsimd.partition_all_reduce(
                                acc[h][:], es[:], P, bass_isa.ReduceOp.add)
                        else:
                            tsum = wsb.tile([P, 512], _F32, tag="tsum",
                                            bufs=6,
                                            name=f"tsum_{lc}_{h}_{kt}")
                            nc.gpsimd.partition_all_reduce(
                                tsum[:, cs], es[:, cs], P,
                                bass_isa.ReduceOp.add)
                            nc.vector.tensor_add(acc[h][:, cs],
                                                 acc[h][:, cs],
                                                 tsum[:, cs])
                # PE-side consumers of es run one key tile behind
                if kt >= 1:
                    for h in hs:
                        av(h, kt - 1)
                    if pe_kt(kt - 1):
                        for h in hs:
                            denom(h, kt - 1)
                pool.pull(3 if lc == 2 else 2)
            for h in hs:
                av(h, nkt - 1)
            if pe_kt(nkt - 1):
                for h in hs:
                    denom(h, nkt - 1)
            for h in hs:
                if psum_sum:
                    nc.vector.tensor_add(acc[h][:], acc[h][:],
                                         psum_sum[h][:])
                rec = wsb.tile([P, 512], _F32, tag="rec", bufs=2,
                               name=f"rec_{lc}_{h}")
                nc.vector.reciprocal(rec[:], acc[h][:])
                nc.vector.tensor_mul(oT[:, h, ls], po[h][:], rec[:])

        def gen_oproj(lt, final=False):
            for mp in range(2):
                ysb = wsb.tile([P, 1024], _BF, tag="ysb", bufs=4,
                               name=f"ysb_{lt}_{mp}")
                pys = []
                for mi in range(2):
                    tag, bufs = (("prj", 2) if mi == 0 else ("ps", 2)) \
                        if final else (f"py{mi}", 1)
                    py_t = psp.tile([P, 512], _F32, tag=tag, bufs=bufs,
                                    name=f"py_{lt}_{mp}_{mi}")
                    pys.append(py_t)
                for h in range(NHL):
                    for mi in range(2):
                        mc = mp * 2 + mi
                        nc.tensor.matmul(
                            pys[mi][:], oT[:, h, lt * P:(lt + 1) * P],
                            wo_sb[:, h, mc * 512:(mc + 1) * 512],
                            start=(h == 0), stop=(h == NHL - 1),
                            skip_group_check=True)
                        yield
                nc.scalar.activation(ysb[:, 0:512], pys[0][:],
                                     mybir.ActivationFunctionType.Copy)
                nc.vector.tensor_copy(ysb[:, 512:1024], pys[1][:])
                nc.sync.dma_start(
                    y_d[lt * P:(lt + 1) * P, mp * 1024:(mp + 1) * 1024],
                    ysb[:])

        class FillerPool:
            """Queue of emission generators, advanced one matmul at a time."""

            def __init__(self, gens):
                self.gens = list(gens)

            def pull(self, n=None):
                while self.gens and (n is None or n > 0):
                    try:
                        next(self.gens[0])
                        if n is not None:
                            n -= 1
                    except StopIteration:
                        self.gens.pop(0)

        # ---- software-pipelined emission
        for et in ET_ORDER:
            FillerPool([gen_proj(0, et)]).pull()
        for lc in range(NLC):
            # in-round fillers: projection matmuls (clean 2-deep PSUM ring);
            # o-proj groups are emitted whole at pair boundaries instead
            # (their single-buffered drains would stall inside rounds),
            # except on the last chunk where they are all that's left.
            if lc + 1 < NLC:
                pool = FillerPool(
                    [gen_proj(lc + 1, et) for et in ET_ORDER])
                boundary = ([gen_oproj(lt)
                             for lt in range(4 * (lc - 1), 4 * lc)]
                            if lc >= 1 else [])
            else:
                pool = FillerPool(
                    [gen_oproj(lt) for lt in range(4 * (lc - 1), 4 * lc)])
                boundary = []
            for slot in range(2):
                for dst, srcap in deferred.get((lc, slot), []):
                    nc.sync.dma_start(dst, srcap)
                emit_att_pair(lc, 2 * slot, pool)
                nb = len(boundary)
                for _ in range(nb):
                    FillerPool([boundary.pop(0)]).pull()
            pool.pull()
        for lt in range(4 * (NLC - 1), 4 * NLC):
            FillerPool([gen_oproj(lt, final=True)]).pull()


def host_constants():
    inv = (1.0 / (10000.0 ** (np.arange(0, HD, 2, dtype=np.float32) / HD))
           ).astype(np.float32)
    t = np.arange(L, dtype=np.float32)
    freqs = t[:, None] * inv[None, :]                    # [L, 64]
    emb = np.concatenate([freqs, freqs], axis=-1)        # [L, 128]
    cosT = np.ascontiguousarray(np.cos(emb).T).astype(BF16)
    sinT = np.ascontiguousarray(np.sin(emb).T).astype(BF16)
    perm = np.zeros((P, P), dtype=F32)
    for i in range(64):
        perm[i + 64, i] = -1.0      # qrot[d] = -q[d+64],  d < 64
        perm[i, i + 64] = 1.0       # qrot[d] =  q[d-64],  d >= 64
    tri = (np.arange(P)[:, None] <= np.arange(P)[None, :]).astype(F32)  # k<=q
    ones = np.ones((P, P), dtype=F32)
    return {
        "cosT": cosT,
        "sinT": sinT,
        "perm": perm.astype(BF16),
        "tri": tri.astype(BF16),
        "ones": ones.astype(BF16),
    }


def make_in_map(consts, x, Wq, Wk, Wv, Wo, b, g):
    qs = slice(g * 512, (g + 1) * 512)
    kvs = slice(g * 128, (g + 1) * 128)
    wq = np.ascontiguousarray(
        Wq[qs].T.reshape(NDT, P, 512).transpose(1, 0, 2)).astype(BF16)
    wk = np.ascontiguousarray(
        Wk[kvs].T.reshape(NDT, P, 128).transpose(1, 0, 2)).astype(BF16)
    wv = np.ascontiguousarray(
        Wv[kvs].T.reshape(NDT, P, 128).transpose(1, 0, 2)).astype(BF16)
    wo = np.ascontiguousarray(
        Wo[:, qs].T.reshape(NHL, P, D).transpose(1, 0, 2)).astype(BF16)
    # x[b].T -> [128, lc, dti, 512]: xh[p, lc, t, j] = x[b, lc*512+j, t*128+p]
    xT = x[b].T                                  # [D, L]
    xh = np.ascontiguousarray(
        xT.reshape(NDT, P, NLC, 512).transpose(1, 2, 0, 3)).astype(BF16)
    return {
        "xh": xh,
        "wq": wq, "wk": wk, "wv": wv, "wo": wo,
        **consts,
    }


_NC_CACHE = {}


def get_nc():
    if "nc" not in _NC_CACHE:
        _NC_CACHE["nc"] = build_nc()
    return _NC_CACHE["nc"]


def kernel(x, Wq, Wk, Wv, Wo):
    x = np.asarray(x, dtype=F32)
    Wq = np.asarray(Wq, dtype=F32)
    Wk = np.asarray(Wk, dtype=F32)
    Wv = np.asarray(Wv, dtype=F32)
    Wo = np.asarray(Wo, dtype=F32)
    nc = get_nc()
    consts = host_constants()
    in_maps = [make_in_map(consts, x, Wq, Wk, Wv, Wo, c // 4, c % 4)
               for c in range(8)]
    res = run_bass_kernel_spmd(nc, in_maps, list(range(8)))
    outs = [np.asarray(r["y"], dtype=F32) for r in res.results]
    y = np.stack([outs[0] + outs[1] + outs[2] + outs[3],
                  outs[4] + outs[5] + outs[6] + outs[7]], axis=0)
    return y
